# revision 12
# baseline (speedup 1.0000x reference)
"""AnyStory Flux attention processor on 8 TRN2 NeuronCores.

Sharding: tensor-parallel over heads (24 heads -> 3 per core), no
collectives; the host gathers along the head axis and performs the final
softmax normalization (divide by the ones-column sums) while unsharding.

Device algorithm per head (S=3168 = 512 txt + 64 redux + 2048 img +
512 ref + 32 router; D=128), in ST orientation (k on partitions, q free):

  seg1 (q 0:2624 x k 0:3136+pad): QK and PV in bf16 (fp8 variants were
    built and measured: fp8 QK adds correlated +-0.1-logit noise on
    dominant keys, and any 8-bit p representation adds >=4% weight
    noise; both blow the 2e-2 tolerance).
    The softmax exp -- the baseline's bottleneck, pinned to the
    Activation engine at ~0.83ns/column -- is split across the only two
    PSUM-capable elementwise engines:
      ACT tiles (13): true exp -> bf16 p (bias aligns the scale to the
        DVE tiles' Schraudolph factor).
      DVE tiles (12, including every masked tile): Schraudolph exp in
        ONE fused tensor op: i16 = round(184.665*x + 16250.43), bitcast
        = bf16 ~ e^x (+-3%); the additive attention masks ride the same
        instruction as the int16 tensor operand
        (i16 = round(184.665*x + am16), am16 = 184.665*mask + 16250.43),
        which removes all mask-matmul work from PE.
  seg2: per-cond ref self-attention, plain bf16 (peaked softmax).
  seg3: router q x [img ; router] keys, bf16, exp on ACT.

  (numerator | denominator) leave PSUM as f16 via ACT/DVE copies
  (alternating) and are DMAed; the host divides while unsharding.
  No max-subtraction anywhere (|logit| <~ 6.6, masks <= 1.5).
"""

import math
import numpy as np
import ml_dtypes
from contextlib import ExitStack

import concourse.bass as bass
import concourse.tile as tile
from concourse import mybir, bacc
from concourse.bass_utils import run_bass_kernel_spmd

# ---- problem constants (hardcoded; kernel.py must be self-contained)
B, H, D = 1, 24, 128
TXT, REDUX, IMG, REF, ROUTER, NCOND = 512, 64, 2048, 512, 32, 2
S = TXT + REDUX + IMG + REF + ROUTER          # 3168
TE = TXT                                       # 512
TRE = TE + REDUX                               # 576
TRI = TRE + IMG                                # 2624
TRIR = TRI + REF                               # 3136
REF_SHIFT = 1.5
SP = 3200                                      # padded key length (25*128)
NKT = SP // 128                                # 25 seg1 k-tiles
HPC = H // 8                                   # heads per core = 3

SQ = 1.0 / math.sqrt(128.0)
A16 = 128.0 / math.log(2.0)                    # 184.664965
B16 = 16250.43                                 # 127*128 - 5.57 (Schraudolph)
# ACT tiles must carry the same scale factor as the Schraudolph tiles:
# bitcast(round(A16*x + B16)) ~ e^x * 2^((B16-16256)/128) * g, E[g]=1.0298
ACT_BIAS = (B16 - 16256.0) / 128.0 * math.log(2.0) + math.log(1.0298)
MASK_CLAMP = -20.0

F32 = mybir.dt.float32
F16 = mybir.dt.float16
BF16 = mybir.dt.bfloat16
I16 = mybir.dt.int16
EXP = mybir.ActivationFunctionType.Exp
COPY = mybir.ActivationFunctionType.Copy
MULT = mybir.AluOpType.mult
ADD = mybir.AluOpType.add

# seg1 q blocks: 384 wide -> 3 PV sub-blocks (3 x 129 cols) per PSUM bank
QBLOCKS = [(0, 384), (384, 384), (768, 384), (1152, 384),
           (1536, 384), (1920, 384), (2304, 320)]
# exp-engine split; D-groups with mask_slot carry the fused additive mask.
# Masked tiles {4, 20..24} sit late in the stream so the am16 chunks have
# streamed in by the time q-block 0 reaches them.
GROUPS = [("A", (0, 1), None), ("D", (14, 15), None),
          ("A", (2, 3), None), ("D", (16, 17), None),
          ("A", (5, 6), None), ("D", (18, 19), None),
          ("A", (7, 8), None), ("D", (4, 20), 0),
          ("A", (9, 10), None), ("D", (21, 22), 2),
          ("A", (11, 12), None), ("D", (23, 24), 4),
          ("A", (13,), None)]


def _subs(qw):
    out, o = [], 0
    while o < qw:
        w = min(128, qw - o)
        out.append((o, w))
        o += w
    return out


def build_nc():
    nc = bacc.Bacc()
    kt_d = nc.declare_dram_parameter("kt", [HPC, 128, SP], BF16, isOutput=False)
    qt_d = nc.declare_dram_parameter("qt", [HPC, 128, S], BF16, isOutput=False)
    # V pre-tiled partition-major [128, T, 129] (value | ones)
    v1_d = nc.declare_dram_parameter("v1", [HPC, 128, NKT, 129], BF16, isOutput=False)
    v2_d = nc.declare_dram_parameter("v2", [HPC, 128, 4, 129], BF16, isOutput=False)
    v3_d = nc.declare_dram_parameter("v3", [HPC, 128, 17, 129], BF16, isOutput=False)
    qt2_d = nc.declare_dram_parameter("qt2", [HPC, 128, 512], BF16, isOutput=False)
    kt2_d = nc.declare_dram_parameter("kt2", [HPC, 128, 512], BF16, isOutput=False)
    # fused-mask payloads: slots (4, 20, 21, 22, 23, 24)
    am_d = nc.declare_dram_parameter("am16", [128, 6, TRI], I16, isOutput=False)
    out_d = nc.declare_dram_parameter("out", [HPC, S, 129], F16, isOutput=True)
    out_f = out_d.rearrange("h s d -> (h s) d")

    with ExitStack() as ctx:
        tc = ctx.enter_context(tile.TileContext(nc))
        const = ctx.enter_context(tc.tile_pool(name="const", bufs=1))
        stp = ctx.enter_context(tc.tile_pool(name="st", bufs=3, space="PSUM"))
        accp = ctx.enter_context(tc.tile_pool(name="acc", bufs=2, space="PSUM"))
        ptp = ctx.enter_context(tc.tile_pool(name="pt", bufs=8))
        stgp = ctx.enter_context(tc.tile_pool(name="stg", bufs=3))
        smallp = ctx.enter_context(tc.tile_pool(name="small", bufs=4))

        am_sb = const.tile([128, 6, TRI], I16, tag="am16")
        bias_sb = const.tile([128, 1], F32, tag="biasA")
        zbias_sb = const.tile([128, 1], F32, tag="biasZ")
        nc.vector.memset(bias_sb[:, :], ACT_BIAS)
        nc.vector.memset(zbias_sb[:, :], 0.0)

        kt_sb, qt_sb, v1_sb = [], [], []
        qt2_sb, kt2_sb, v2_sb, v3_sb = [], [], [], []
        # kt column ranges per group, in consumption order
        GCOLS = []
        for eng, tiles, mslot in GROUPS:
            lo, hi = min(tiles) * 128, (max(tiles) + 1) * 128
            if hi - lo == len(tiles) * 128:
                GCOLS.append([(lo, hi)])
            else:
                GCOLS.append([(t * 128, (t + 1) * 128) for t in tiles])
        for h in range(HPC):
            kt = const.tile([128, SP], BF16, tag=f"kt{h}")
            qt = const.tile([128, S], BF16, tag=f"qt{h}")
            v1 = const.tile([128, NKT, 129], BF16, tag=f"v1{h}")
            qt2 = const.tile([128, 512], BF16, tag=f"qt2{h}")
            kt2 = const.tile([128, 512], BF16, tag=f"kt2{h}")
            v2 = const.tile([128, 4, 129], BF16, tag=f"v2{h}")
            v3 = const.tile([128, 17, 129], BF16, tag=f"v3{h}")
            kt_sb.append(kt); qt_sb.append(qt); v1_sb.append(v1)
            qt2_sb.append(qt2); kt2_sb.append(kt2)
            v2_sb.append(v2); v3_sb.append(v3)

            def ktg(gi, kt=kt, h=h):
                for c0, c1 in GCOLS[gi]:
                    nc.sync.dma_start(kt[:, c0:c1], kt_d[h, :, c0:c1])

            def qtc(c0, c1, qt=qt, h=h):
                nc.sync.dma_start(qt[:, c0:c1], qt_d[h, :, c0:c1])

            def amc(c0, c1):
                nc.sync.dma_start(am_sb[:, :, c0:c1], am_d[:, :, c0:c1])

            def v1c(t0, t1, v1=v1, h=h):
                nc.sync.dma_start(v1[:, t0:t1, :], v1_d[h, :, t0:t1, :])

            if h == 0:
                # JIT order: chunks land just before the pipeline consumes
                # them (kt per group, qt per q-block, masks before q-block
                # 0's masked groups, V ~two groups behind the exp stream)
                ktg(0); qtc(0, 384); ktg(1); v1c(0, 4)
                ktg(2); v1c(14, 20); ktg(3); amc(0, 384)
                ktg(4); ktg(5); v1c(4, 14); ktg(6); ktg(7)
                ktg(8); v1c(20, 25); ktg(9); ktg(10)
                ktg(11); ktg(12)
                qtc(384, 768); amc(384, 1152)
                qtc(768, 1536); amc(1152, 2048)
                qtc(1536, 2304); amc(2048, TRI)
                qtc(2304, S)
            else:
                for c in range(4):
                    nc.sync.dma_start(kt[:, c * 800:(c + 1) * 800],
                                      kt_d[h, :, c * 800:(c + 1) * 800])
                    q0c, q1c = c * 792, min((c + 1) * 792, S)
                    nc.sync.dma_start(qt[:, q0c:q1c], qt_d[h, :, q0c:q1c])
                for c in range(5):
                    v1c(c * 5, min((c + 1) * 5, NKT))
            nc.sync.dma_start(qt2[:, :], qt2_d[h])
            nc.sync.dma_start(kt2[:, :], kt2_d[h])
            nc.sync.dma_start(v2[:, :, :], v2_d[h])
            nc.sync.dma_start(v3[:, :, :], v3_d[h])

        # ---- work items (global 2-deep software pipeline)
        items = []
        copy_flip = [0]

        def finalize(acc, h, q0, qw):
            """Copy acc PSUM -> f16 stage (ACT/DVE alternating), DMA out."""
            subs = _subs(qw)
            stg = stgp.tile([128, 3, 129], F16, tag="stg", name="stg")
            on_act = (copy_flip[0] % 2 == 0)
            copy_flip[0] += 1
            nsub = len(subs)
            if all(wsub == 128 for _, wsub in subs):
                src = acc[:, 0:nsub * 129].rearrange("p (a b) -> p a b", a=nsub)
                if on_act:
                    nc.scalar.activation(stg[:, 0:nsub, :], src, COPY)
                else:
                    nc.vector.tensor_scalar_add(stg[:, 0:nsub, :], src, 0.0)
            else:
                # ragged tail: avoid reading never-written PSUM
                w_full = (nsub - 1) * 129
                lastw = subs[-1][1]
                if nsub > 1:
                    src = acc[:, 0:w_full].rearrange("p (a b) -> p a b", a=nsub - 1)
                    if on_act:
                        nc.scalar.activation(stg[:, 0:nsub - 1, :], src, COPY)
                    else:
                        nc.vector.tensor_scalar_add(stg[:, 0:nsub - 1, :], src, 0.0)
                if on_act:
                    nc.scalar.activation(stg[0:lastw, nsub - 1, :],
                                         acc[0:lastw, w_full:w_full + 129], COPY)
                else:
                    nc.vector.tensor_scalar_add(stg[0:lastw, nsub - 1, :],
                                                acc[0:lastw, w_full:w_full + 129], 0.0)
            for si, (qs0, qsw) in enumerate(subs):
                r0 = h * S + q0 + qs0
                nc.sync.dma_start(out_f[r0:r0 + qsw, :], stg[0:qsw, si, :])

        for h in range(HPC):
            kt, qt, v1 = kt_sb[h], qt_sb[h], v1_sb[h]
            head_items = []

            for qbi, (q0, qw) in enumerate(QBLOCKS):
                subs = _subs(qw)
                blk = {}

                def qk1(st, gi, kt=kt, qt=qt, q0=q0, qw=qw):
                    for j, t in enumerate(GROUPS[gi][1]):
                        nc.tensor.matmul(
                            st[:, j, 0:qw],
                            lhsT=kt[:, t * 128:(t + 1) * 128],
                            rhs=qt[:, q0:q0 + qw],
                            start=True, stop=True)

                def ex1(st, gi, q0=q0, qw=qw):
                    eng, tiles, mslot = GROUPS[gi]
                    n = len(tiles)
                    pt = ptp.tile([128, 2, 512], BF16, tag="pt", name="pt")
                    if eng == "A":
                        nc.scalar.activation(pt[:, 0:n, 0:qw], st[:, 0:n, 0:qw],
                                             EXP, bias=bias_sb[:, :], scale=1.0)
                    elif mslot is None:
                        nc.vector.tensor_scalar(
                            pt[:, 0:n, 0:qw].bitcast(I16),
                            st[:, 0:n, 0:qw], A16, B16, MULT, ADD)
                    else:
                        nc.vector.scalar_tensor_tensor(
                            pt[:, 0:n, 0:qw].bitcast(I16),
                            st[:, 0:n, 0:qw], A16,
                            am_sb[:, mslot:mslot + n, q0:q0 + qw], MULT, ADD)
                    return pt

                def pv1(pt, gi, h=h, v1=v1, q0=q0, qw=qw, subs=subs, blk=blk):
                    if "acc" not in blk:
                        blk["acc"] = accp.tile([128, 512], F32, tag="acc", name="acc")
                        blk["n"] = 0
                    acc = blk["acc"]
                    tiles = GROUPS[gi][1]
                    last_group = (gi == len(GROUPS) - 1)
                    for j, t in enumerate(tiles):
                        for si, (qs0, qsw) in enumerate(subs):
                            nc.tensor.matmul(
                                acc[0:qsw, si * 129:si * 129 + 129],
                                lhsT=pt[:, j, qs0:qs0 + qsw],
                                rhs=v1[:, t, :],
                                start=(blk["n"] == 0),
                                stop=(last_group and j == len(tiles) - 1
                                      and si == len(subs) - 1))
                            blk["n"] += 1
                    if last_group:
                        return lambda: finalize(acc, h, q0, qw)

                for gi in range(len(GROUPS)):
                    head_items.append((
                        (lambda st, g=gi, f=qk1: f(st, g)),
                        (lambda st, g=gi, f=ex1: f(st, g)),
                        (lambda pt, g=gi, f=pv1: f(pt, g)),
                    ))

            # ===== seg2: per-cond ref self-attention (bf16) =====
            seg23_items = []
            for c in range(NCOND):
                b0 = 256 * c

                def qk2(st, h=h, b0=b0):
                    for j in range(2):
                        nc.tensor.matmul(
                            st[:, j, 0:256],
                            lhsT=kt2_sb[h][:, b0 + j * 128:b0 + (j + 1) * 128],
                            rhs=qt2_sb[h][:, b0:b0 + 256],
                            start=True, stop=True)

                def ex2(st, h=h):
                    pt = smallp.tile([128, 2, 256], BF16, tag="pt2", name="pt2")
                    nc.scalar.activation(pt[:, 0:2, 0:256], st[:, 0:2, 0:256],
                                         EXP, bias=zbias_sb[:, :], scale=1.0)
                    return pt

                def pv2(pt, h=h, b0=b0, c=c):
                    acc = accp.tile([128, 512], F32, tag="acc", name="acc")
                    for j in range(2):
                        for si in range(2):
                            nc.tensor.matmul(
                                acc[0:128, si * 129:si * 129 + 129],
                                lhsT=pt[:, j, si * 128:(si + 1) * 128],
                                rhs=v2_sb[h][:, 2 * c + j, :],
                                start=(j == 0 and si == 0),
                                stop=(j == 1 and si == 1))
                    return lambda: finalize(acc, h, TRI + b0, 256)

                seg23_items.append((qk2, ex2, pv2))

            # ===== seg3: router queries =====
            def qk3(st, h=h, kt=kt, qt=qt):
                for i in range(16):
                    nc.tensor.matmul(
                        st[:, 0, i * 32:(i + 1) * 32],
                        lhsT=kt[:, TRE + i * 128:TRE + (i + 1) * 128],
                        rhs=qt[:, TRIR:TRIR + 32],
                        start=(i == 0), stop=(i == 15))
                nc.tensor.matmul(
                    st[0:32, 1, 0:32],
                    lhsT=kt[:, TRIR:TRIR + 32],
                    rhs=qt[:, TRIR:TRIR + 32],
                    start=True, stop=True)

            def ex3(st, h=h):
                pt = smallp.tile([128, 2, 512], BF16, tag="pt3", name="pt3")
                nc.scalar.activation(pt[:, 0, 0:512], st[:, 0, 0:512],
                                     EXP, bias=zbias_sb[:, :], scale=1.0)
                nc.scalar.activation(pt[0:32, 1, 0:32], st[0:32, 1, 0:32],
                                     EXP, bias=zbias_sb[0:32, :], scale=1.0)
                return pt

            def pv3(pt, h=h):
                acc = accp.tile([128, 512], F32, tag="acc", name="acc")
                for i in range(16):
                    nc.tensor.matmul(
                        acc[0:32, 0:129],
                        lhsT=pt[:, 0, i * 32:(i + 1) * 32],
                        rhs=v3_sb[h][:, i, :],
                        start=(i == 0), stop=False)
                nc.tensor.matmul(
                    acc[0:32, 0:129],
                    lhsT=pt[0:32, 1, 0:32],
                    rhs=v3_sb[h][0:32, 16, :],
                    start=False, stop=True)
                return lambda: finalize(acc, h, TRIR, 32)

            seg23_items.append((qk3, ex3, pv3))
            # splice seg2/3 into the middle of the head's stream so their
            # small bursty windows don't cluster at head boundaries
            for i, it in enumerate(seg23_items):
                head_items.insert(30 + i * 13, it)
            items.extend(head_items)

        # ---- run the global pipeline; finalize copies are deferred a few
        # items so they never sit in an exp engine's queue ahead of work the
        # PE pipeline depends on
        pending, fins = [], []
        idx = 0
        for (fqk, fex, fpv) in items:
            while fins and fins[0][1] <= idx:
                fins.pop(0)[0]()
            st = stp.tile([128, 2, 512], F32, tag="st", name="st")
            fqk(st)
            while len(pending) >= 2:
                fin = pending.pop(0)()
                if fin is not None:
                    fins.append((fin, idx + 4))
            pt = fex(st)
            pending.append(lambda f=fpv, p=pt: f(p))
            idx += 1
        while pending:
            fin = pending.pop(0)()
            if fin is not None:
                fins.append((fin, 0))
        while fins:
            fins.pop(0)[0]()

    nc.compile()
    return nc


_NC_CACHE = None


def _get_nc():
    global _NC_CACHE
    if _NC_CACHE is None:
        _NC_CACHE = build_nc()
    return _NC_CACHE


def make_in_maps(query, key, value, ref_mask, routing_map):
    q = np.asarray(query, np.float32)[0]                  # [24, S, 128]
    k = np.asarray(key, np.float32)[0]
    v = np.asarray(value, np.float32)[0]
    rm = np.asarray(ref_mask, np.float32)[0]              # [512, 2624]
    rt = np.asarray(routing_map, np.float32)[0]           # [2, 2048]

    qt = np.ascontiguousarray(
        (q * SQ).transpose(0, 2, 1)).astype(ml_dtypes.bfloat16)   # [24,128,S]
    ktf = np.zeros((H, 128, SP), np.float32)
    ktf[:, :, :S] = k.transpose(0, 2, 1)
    kt = ktf.astype(ml_dtypes.bfloat16)

    # V (+ones) pre-tiled partition-major [128, T, 129]
    vv = np.zeros((H, SP, 129), np.float32)
    vv[:, :S, :128] = v
    vv[:, :TRIR, 128] = 1.0                               # ones: seg1 keys only
    vv[:, 24 * 128 + 64:] = 0.0                           # router+pad rows
    v1 = np.ascontiguousarray(
        vv.reshape(H, NKT, 128, 129).transpose(0, 2, 1, 3)).astype(ml_dtypes.bfloat16)

    v2 = np.zeros((H, 128, 4, 129), np.float32)
    for j in range(4):
        v2[:, :, j, :128] = v[:, TRI + j * 128:TRI + (j + 1) * 128]
        v2[:, :, j, 128] = 1.0
    v2 = v2.astype(ml_dtypes.bfloat16)
    v3 = np.zeros((H, 128, 17, 129), np.float32)
    for i in range(16):
        t0 = TRE + i * 128
        v3[:, :, i, :128] = v[:, t0:t0 + 128]
        v3[:, :, i, 128] = 1.0
    v3[:, 0:32, 16, :128] = v[:, TRIR:S]
    v3[:, 0:32, 16, 128] = 1.0
    v3 = v3.astype(ml_dtypes.bfloat16)

    qt2 = np.ascontiguousarray(
        (q[:, TRI:TRIR] * SQ).transpose(0, 2, 1)).astype(ml_dtypes.bfloat16)
    kt2 = np.ascontiguousarray(
        k[:, TRI:TRIR].transpose(0, 2, 1)).astype(ml_dtypes.bfloat16)

    # fused-mask payloads am16 = round(A16*mask + B16), slots (4,20..24)
    M = (rm - 1.0) * 100.0 + REF_SHIFT                    # [512, 2624]
    ref_rt = np.repeat(rt, REF // NCOND, axis=0)
    M[:, TRE:TRI] += (ref_rt - 1.0) * 100.0
    M = np.maximum(M, MASK_CLAMP)
    redux_m = np.maximum((rt - 1.0) * 100.0, MASK_CLAMP)  # [2, 2048]
    am = np.zeros((6, 128, TRI), np.float32)
    am[0, 0:32, TRE:TRI] = A16 * redux_m[0][None, :]      # tile 4 rows: redux
    am[0, 32:64, TRE:TRI] = A16 * redux_m[1][None, :]
    for tt in range(5):                                   # tiles 20..24
        blkm = np.zeros((128, TRI), np.float32)
        kk0 = (20 + tt) * 128 - TRI                       # ref-relative row
        for r in range(128):
            kr = kk0 + r
            if 0 <= kr < REF:
                blkm[r] = A16 * M[kr]
            elif kr >= REF:
                blkm[r] = A16 * MASK_CLAMP                # router+pad rows
        am[1 + tt] = blkm
    am16 = np.round(am + B16).astype(np.int16)
    am16 = np.ascontiguousarray(am16.transpose(1, 0, 2))  # [128, 6, TRI]

    in_maps = []
    for cc in range(8):
        hs = slice(HPC * cc, HPC * (cc + 1))
        in_maps.append({
            "kt": np.ascontiguousarray(kt[hs]),
            "qt": np.ascontiguousarray(qt[hs]),
            "v1": np.ascontiguousarray(v1[hs]),
            "v2": np.ascontiguousarray(v2[hs]),
            "v3": np.ascontiguousarray(v3[hs]),
            "qt2": np.ascontiguousarray(qt2[hs]),
            "kt2": np.ascontiguousarray(kt2[hs]),
            "am16": am16,
        })
    return in_maps


def kernel(query, key, value, ref_mask, routing_map, **_ignored):
    import jax
    if not any(d.platform == "axon" for d in jax.devices()):
        jax.config.update("jax_platforms", "axon,cpu")
    nc = _get_nc()
    in_maps = make_in_maps(query, key, value, ref_mask, routing_map)
    res = run_bass_kernel_spmd(nc, in_maps, core_ids=list(range(8)))
    outs = [res.results[c]["out"] for c in range(8)]      # [3, S, 129] f16
    full = np.concatenate(outs, axis=0).astype(np.float32)
    out = full[:, :, :128] / full[:, :, 128:129]
    return np.ascontiguousarray(out[None].astype(np.float32))


# revision 14
# speedup vs baseline: 1.0396x; 1.0396x over previous
"""AnyStory Flux attention processor on 8 TRN2 NeuronCores.

Sharding: tensor-parallel over heads (24 heads -> 3 per core), no
collectives; the host gathers along the head axis and performs the final
softmax normalization (divide by the ones-column sums) while unsharding.

Device algorithm per head (S=3168 = 512 txt + 64 redux + 2048 img +
512 ref + 32 router; D=128), in ST orientation (k on partitions, q free):

  seg1 (q 0:2624 x k 0:3136+pad): QK and PV in bf16 (fp8 variants were
    built and measured: fp8 QK adds correlated +-0.1-logit noise on
    dominant keys, and any 8-bit p representation adds >=4% weight
    noise; both blow the 2e-2 tolerance).
    The softmax exp -- the baseline's bottleneck, pinned to the
    Activation engine at ~0.83ns/column -- is split across the only two
    PSUM-capable elementwise engines:
      ACT tiles (13): true exp -> bf16 p (bias aligns the scale to the
        DVE tiles' Schraudolph factor).
      DVE tiles (12, including every masked tile): Schraudolph exp in
        ONE fused tensor op: i16 = round(184.665*x + 16250.43), bitcast
        = bf16 ~ e^x (+-3%); the additive attention masks ride the same
        instruction as the int16 tensor operand
        (i16 = round(184.665*x + am16), am16 = 184.665*mask + 16250.43),
        which removes all mask-matmul work from PE.
  seg2: per-cond ref self-attention, plain bf16 (peaked softmax).
  seg3: router q x [img ; router] keys, bf16, exp on ACT.

  (numerator | denominator) leave PSUM as f16 via ACT/DVE copies
  (alternating) and are DMAed; the host divides while unsharding.
  No max-subtraction anywhere (|logit| <~ 6.6, masks <= 1.5).
"""

import math
import numpy as np
import ml_dtypes
from contextlib import ExitStack

import concourse.bass as bass
import concourse.tile as tile
from concourse import mybir, bacc
from concourse.bass_utils import run_bass_kernel_spmd

# ---- problem constants (hardcoded; kernel.py must be self-contained)
B, H, D = 1, 24, 128
TXT, REDUX, IMG, REF, ROUTER, NCOND = 512, 64, 2048, 512, 32, 2
S = TXT + REDUX + IMG + REF + ROUTER          # 3168
TE = TXT                                       # 512
TRE = TE + REDUX                               # 576
TRI = TRE + IMG                                # 2624
TRIR = TRI + REF                               # 3136
REF_SHIFT = 1.5
SP = 3200                                      # padded key length (25*128)
NKT = SP // 128                                # 25 seg1 k-tiles
HPC = H // 8                                   # heads per core = 3

SQ = 1.0 / math.sqrt(128.0)
A16 = 128.0 / math.log(2.0)                    # 184.664965
B16 = 16250.43                                 # 127*128 - 5.57 (Schraudolph)
# ACT tiles must carry the same scale factor as the Schraudolph tiles:
# bitcast(round(A16*x + B16)) ~ e^x * 2^((B16-16256)/128) * g, E[g]=1.0298
ACT_BIAS = (B16 - 16256.0) / 128.0 * math.log(2.0) + math.log(1.0298)
MASK_CLAMP = -20.0

F32 = mybir.dt.float32
F16 = mybir.dt.float16
BF16 = mybir.dt.bfloat16
I16 = mybir.dt.int16
EXP = mybir.ActivationFunctionType.Exp
COPY = mybir.ActivationFunctionType.Copy
MULT = mybir.AluOpType.mult
ADD = mybir.AluOpType.add

# seg1 q blocks: 384 wide -> 3 PV sub-blocks (3 x 129 cols) per PSUM bank
QBLOCKS = [(0, 384), (384, 384), (768, 384), (1152, 384),
           (1536, 384), (1920, 384), (2304, 320)]
# exp-engine split; D-groups with mask_slot carry the fused additive mask.
# Masked tiles {4, 20..24} sit late in the stream so the am16 chunks have
# streamed in by the time q-block 0 reaches them.
GROUPS = [("A", (0, 1), None), ("D", (14, 15), None),
          ("A", (2, 3), None), ("D", (16, 17), None),
          ("A", (5, 6), None), ("D", (18, 19), None),
          ("A", (7, 8), None), ("D", (4, 20), 0),
          ("A", (9, 10), None), ("D", (21, 22), 2),
          ("A", (11, 12), None), ("D", (23, 24), 4),
          ("A", (13,), None)]


def _subs(qw):
    out, o = [], 0
    while o < qw:
        w = min(128, qw - o)
        out.append((o, w))
        o += w
    return out


def build_nc():
    nc = bacc.Bacc()
    kt_d = nc.declare_dram_parameter("kt", [HPC, 128, SP], BF16, isOutput=False)
    qt_d = nc.declare_dram_parameter("qt", [HPC, 128, S], BF16, isOutput=False)
    # V pre-tiled partition-major [128, T, 129] (value | ones)
    v1_d = nc.declare_dram_parameter("v1", [HPC, 128, NKT, 129], BF16, isOutput=False)
    v2_d = nc.declare_dram_parameter("v2", [HPC, 128, 4, 129], BF16, isOutput=False)
    v3_d = nc.declare_dram_parameter("v3", [HPC, 128, 17, 129], BF16, isOutput=False)
    qt2_d = nc.declare_dram_parameter("qt2", [HPC, 128, 512], BF16, isOutput=False)
    kt2_d = nc.declare_dram_parameter("kt2", [HPC, 128, 512], BF16, isOutput=False)
    # fused-mask payloads: slots (4, 20, 21, 22, 23, 24)
    am_d = nc.declare_dram_parameter("am16", [128, 6, TRI], I16, isOutput=False)
    out_d = nc.declare_dram_parameter("out", [HPC, S, 129], F16, isOutput=True)
    out_f = out_d.rearrange("h s d -> (h s) d")

    with ExitStack() as ctx:
        tc = ctx.enter_context(tile.TileContext(nc))
        const = ctx.enter_context(tc.tile_pool(name="const", bufs=1))
        stp = ctx.enter_context(tc.tile_pool(name="st", bufs=3, space="PSUM"))
        accp = ctx.enter_context(tc.tile_pool(name="acc", bufs=2, space="PSUM"))
        ptp = ctx.enter_context(tc.tile_pool(name="pt", bufs=8))
        stgp = ctx.enter_context(tc.tile_pool(name="stg", bufs=10))
        smallp = ctx.enter_context(tc.tile_pool(name="small", bufs=4))

        am_sb = const.tile([128, 6, TRI], I16, tag="am16")
        bias_sb = const.tile([128, 1], F32, tag="biasA")
        zbias_sb = const.tile([128, 1], F32, tag="biasZ")
        nc.vector.memset(bias_sb[:, :], ACT_BIAS)
        nc.vector.memset(zbias_sb[:, :], 0.0)

        kt_sb, qt_sb, v1_sb = [], [], []
        qt2_sb, kt2_sb, v2_sb, v3_sb = [], [], [], []
        # kt column ranges per group, in consumption order
        GCOLS = []
        for eng, tiles, mslot in GROUPS:
            lo, hi = min(tiles) * 128, (max(tiles) + 1) * 128
            if hi - lo == len(tiles) * 128:
                GCOLS.append([(lo, hi)])
            else:
                GCOLS.append([(t * 128, (t + 1) * 128) for t in tiles])
        load_fns = []
        for h in range(HPC):
            kt = const.tile([128, SP], BF16, tag=f"kt{h}")
            qt = const.tile([128, S], BF16, tag=f"qt{h}")
            v1 = const.tile([128, NKT, 129], BF16, tag=f"v1{h}")
            qt2 = const.tile([128, 512], BF16, tag=f"qt2{h}")
            kt2 = const.tile([128, 512], BF16, tag=f"kt2{h}")
            v2 = const.tile([128, 4, 129], BF16, tag=f"v2{h}")
            v3 = const.tile([128, 17, 129], BF16, tag=f"v3{h}")
            kt_sb.append(kt); qt_sb.append(qt); v1_sb.append(v1)
            qt2_sb.append(qt2); kt2_sb.append(kt2)
            v2_sb.append(v2); v3_sb.append(v3)

            def load_head(h=h, kt=kt, qt=qt, v1=v1, qt2=qt2, kt2=kt2,
                          v2=v2, v3=v3):
                def ktg(gi):
                    for c0, c1 in GCOLS[gi]:
                        nc.sync.dma_start(kt[:, c0:c1], kt_d[h, :, c0:c1])

                def qtc(c0, c1):
                    nc.sync.dma_start(qt[:, c0:c1], qt_d[h, :, c0:c1])

                def amc(c0, c1):
                    nc.sync.dma_start(am_sb[:, :, c0:c1], am_d[:, :, c0:c1])

                def v1c(t0, t1):
                    nc.sync.dma_start(v1[:, t0:t1, :], v1_d[h, :, t0:t1, :])

                if h == 0:
                    # JIT order: chunks land just before the pipeline
                    # consumes them (kt per group, qt per q-block, masks
                    # before q-block 0's masked groups)
                    ktg(0); qtc(0, 384); ktg(1); v1c(0, 4)
                    ktg(2); v1c(14, 20); ktg(3); amc(0, 384)
                    ktg(4); ktg(5); v1c(4, 14); ktg(6); ktg(7)
                    ktg(8); v1c(20, 25); ktg(9); ktg(10)
                    ktg(11); ktg(12)
                    qtc(384, 768); amc(384, 1152)
                    qtc(768, 1536); amc(1152, 2048)
                    qtc(1536, 2304); amc(2048, TRI)
                    qtc(2304, S)
                else:
                    for c in range(4):
                        nc.sync.dma_start(kt[:, c * 800:(c + 1) * 800],
                                          kt_d[h, :, c * 800:(c + 1) * 800])
                        q0c, q1c = c * 792, min((c + 1) * 792, S)
                        nc.sync.dma_start(qt[:, q0c:q1c], qt_d[h, :, q0c:q1c])
                    for c in range(5):
                        v1c(c * 5, min((c + 1) * 5, NKT))
                nc.sync.dma_start(qt2[:, :], qt2_d[h])
                nc.sync.dma_start(kt2[:, :], kt2_d[h])
                nc.sync.dma_start(v2[:, :, :], v2_d[h])
                nc.sync.dma_start(v3[:, :, :], v3_d[h])
            load_fns.append(load_head)

        # ---- work items (global 2-deep software pipeline)
        items = []
        copy_flip = [0]

        def finalize(acc, h, q0, qw):
            """Copy acc PSUM -> f16 stage (ACT/DVE alternating), DMA out."""
            subs = _subs(qw)
            stg = stgp.tile([128, 3, 129], F16, tag="stg", name="stg")
            on_act = (copy_flip[0] % 2 == 0)
            copy_flip[0] += 1
            nsub = len(subs)
            if all(wsub == 128 for _, wsub in subs):
                src = acc[:, 0:nsub * 129].rearrange("p (a b) -> p a b", a=nsub)
                if on_act:
                    nc.scalar.activation(stg[:, 0:nsub, :], src, COPY)
                else:
                    nc.vector.tensor_scalar_add(stg[:, 0:nsub, :], src, 0.0)
            else:
                # ragged tail: avoid reading never-written PSUM
                w_full = (nsub - 1) * 129
                lastw = subs[-1][1]
                if nsub > 1:
                    src = acc[:, 0:w_full].rearrange("p (a b) -> p a b", a=nsub - 1)
                    if on_act:
                        nc.scalar.activation(stg[:, 0:nsub - 1, :], src, COPY)
                    else:
                        nc.vector.tensor_scalar_add(stg[:, 0:nsub - 1, :], src, 0.0)
                if on_act:
                    nc.scalar.activation(stg[0:lastw, nsub - 1, :],
                                         acc[0:lastw, w_full:w_full + 129], COPY)
                else:
                    nc.vector.tensor_scalar_add(stg[0:lastw, nsub - 1, :],
                                                acc[0:lastw, w_full:w_full + 129], 0.0)
            r0 = h * S + q0
            nfull = qw // 128
            if nfull:
                dst = out_f[r0:r0 + nfull * 128, :].rearrange(
                    "(si p) d -> p si d", si=nfull)
                nc.sync.dma_start(dst, stg[:, 0:nfull, :])
            if qw % 128:
                lw = qw % 128
                nc.sync.dma_start(out_f[r0 + nfull * 128:r0 + qw, :],
                                  stg[0:lw, nfull, :])

        for h in range(HPC):
            load_fns[h]()
            kt, qt, v1 = kt_sb[h], qt_sb[h], v1_sb[h]
            head_items = []

            for qbi, (q0, qw) in enumerate(QBLOCKS):
                subs = _subs(qw)
                blk = {}

                def qk1(st, gi, kt=kt, qt=qt, q0=q0, qw=qw):
                    for j, t in enumerate(GROUPS[gi][1]):
                        nc.tensor.matmul(
                            st[:, j, 0:qw],
                            lhsT=kt[:, t * 128:(t + 1) * 128],
                            rhs=qt[:, q0:q0 + qw],
                            start=True, stop=True)

                def ex1(st, gi, q0=q0, qw=qw):
                    eng, tiles, mslot = GROUPS[gi]
                    n = len(tiles)
                    pt = ptp.tile([128, 2, 512], BF16, tag="pt", name="pt")
                    if eng == "A":
                        nc.scalar.activation(pt[:, 0:n, 0:qw], st[:, 0:n, 0:qw],
                                             EXP, bias=bias_sb[:, :], scale=1.0)
                    elif mslot is None:
                        nc.vector.tensor_scalar(
                            pt[:, 0:n, 0:qw].bitcast(I16),
                            st[:, 0:n, 0:qw], A16, B16, MULT, ADD)
                    else:
                        nc.vector.scalar_tensor_tensor(
                            pt[:, 0:n, 0:qw].bitcast(I16),
                            st[:, 0:n, 0:qw], A16,
                            am_sb[:, mslot:mslot + n, q0:q0 + qw], MULT, ADD)
                    return pt

                def pv1(pt, gi, h=h, v1=v1, q0=q0, qw=qw, subs=subs, blk=blk):
                    if "acc" not in blk:
                        blk["acc"] = accp.tile([128, 512], F32, tag="acc", name="acc")
                        blk["n"] = 0
                    acc = blk["acc"]
                    tiles = GROUPS[gi][1]
                    last_group = (gi == len(GROUPS) - 1)
                    for j, t in enumerate(tiles):
                        for si, (qs0, qsw) in enumerate(subs):
                            nc.tensor.matmul(
                                acc[0:qsw, si * 129:si * 129 + 129],
                                lhsT=pt[:, j, qs0:qs0 + qsw],
                                rhs=v1[:, t, :],
                                start=(blk["n"] == 0),
                                stop=(last_group and j == len(tiles) - 1
                                      and si == len(subs) - 1))
                            blk["n"] += 1
                    if last_group:
                        return lambda: finalize(acc, h, q0, qw)

                for gi in range(len(GROUPS)):
                    head_items.append((
                        (lambda st, g=gi, f=qk1: f(st, g)),
                        (lambda st, g=gi, f=ex1: f(st, g)),
                        (lambda pt, g=gi, f=pv1: f(pt, g)),
                    ))

            # ===== seg2: per-cond ref self-attention (bf16) =====
            seg23_items = []
            for c in range(NCOND):
                b0 = 256 * c

                def qk2(st, h=h, b0=b0):
                    for j in range(2):
                        nc.tensor.matmul(
                            st[:, j, 0:256],
                            lhsT=kt2_sb[h][:, b0 + j * 128:b0 + (j + 1) * 128],
                            rhs=qt2_sb[h][:, b0:b0 + 256],
                            start=True, stop=True)

                def ex2(st, h=h):
                    pt = smallp.tile([128, 2, 256], BF16, tag="pt2", name="pt2")
                    nc.scalar.activation(pt[:, 0:2, 0:256], st[:, 0:2, 0:256],
                                         EXP, bias=zbias_sb[:, :], scale=1.0)
                    return pt

                def pv2(pt, h=h, b0=b0, c=c):
                    acc = accp.tile([128, 512], F32, tag="acc", name="acc")
                    for j in range(2):
                        for si in range(2):
                            nc.tensor.matmul(
                                acc[0:128, si * 129:si * 129 + 129],
                                lhsT=pt[:, j, si * 128:(si + 1) * 128],
                                rhs=v2_sb[h][:, 2 * c + j, :],
                                start=(j == 0 and si == 0),
                                stop=(j == 1 and si == 1))
                    return lambda: finalize(acc, h, TRI + b0, 256)

                seg23_items.append((qk2, ex2, pv2))

            # ===== seg3: router queries =====
            def qk3(st, h=h, kt=kt, qt=qt):
                for i in range(16):
                    nc.tensor.matmul(
                        st[:, 0, i * 32:(i + 1) * 32],
                        lhsT=kt[:, TRE + i * 128:TRE + (i + 1) * 128],
                        rhs=qt[:, TRIR:TRIR + 32],
                        start=(i == 0), stop=(i == 15))
                nc.tensor.matmul(
                    st[0:32, 1, 0:32],
                    lhsT=kt[:, TRIR:TRIR + 32],
                    rhs=qt[:, TRIR:TRIR + 32],
                    start=True, stop=True)

            def ex3(st, h=h):
                pt = smallp.tile([128, 2, 512], BF16, tag="pt3", name="pt3")
                nc.scalar.activation(pt[:, 0, 0:512], st[:, 0, 0:512],
                                     EXP, bias=zbias_sb[:, :], scale=1.0)
                nc.scalar.activation(pt[0:32, 1, 0:32], st[0:32, 1, 0:32],
                                     EXP, bias=zbias_sb[0:32, :], scale=1.0)
                return pt

            def pv3(pt, h=h):
                acc = accp.tile([128, 512], F32, tag="acc", name="acc")
                for i in range(16):
                    nc.tensor.matmul(
                        acc[0:32, 0:129],
                        lhsT=pt[:, 0, i * 32:(i + 1) * 32],
                        rhs=v3_sb[h][:, i, :],
                        start=(i == 0), stop=False)
                nc.tensor.matmul(
                    acc[0:32, 0:129],
                    lhsT=pt[0:32, 1, 0:32],
                    rhs=v3_sb[h][0:32, 16, :],
                    start=False, stop=True)
                return lambda: finalize(acc, h, TRIR, 32)

            seg23_items.append((qk3, ex3, pv3))
            # splice seg2/3 into the middle of the head's stream so their
            # small bursty windows don't cluster at head boundaries
            for i, it in enumerate(seg23_items):
                head_items.insert(30 + i * 13, it)
            items.extend(head_items)

        # ---- run the global pipeline; finalize copies are deferred a few
        # items so they never sit in an exp engine's queue ahead of work the
        # PE pipeline depends on
        pending, fins = [], []
        idx = 0
        for (fqk, fex, fpv) in items:
            while fins and fins[0][1] <= idx:
                fins.pop(0)[0]()
            st = stp.tile([128, 2, 512], F32, tag="st", name="st")
            fqk(st)
            while len(pending) >= 2:
                fin = pending.pop(0)()
                if fin is not None:
                    fins.append((fin, idx + 4))
            pt = fex(st)
            pending.append(lambda f=fpv, p=pt: f(p))
            idx += 1
        while pending:
            fin = pending.pop(0)()
            if fin is not None:
                fins.append((fin, 0))
        while fins:
            fins.pop(0)[0]()

    nc.compile()
    return nc


_NC_CACHE = None


def _get_nc():
    global _NC_CACHE
    if _NC_CACHE is None:
        _NC_CACHE = build_nc()
    return _NC_CACHE


def make_in_maps(query, key, value, ref_mask, routing_map):
    q = np.asarray(query, np.float32)[0]                  # [24, S, 128]
    k = np.asarray(key, np.float32)[0]
    v = np.asarray(value, np.float32)[0]
    rm = np.asarray(ref_mask, np.float32)[0]              # [512, 2624]
    rt = np.asarray(routing_map, np.float32)[0]           # [2, 2048]

    qt = np.ascontiguousarray(
        (q * SQ).transpose(0, 2, 1)).astype(ml_dtypes.bfloat16)   # [24,128,S]
    ktf = np.zeros((H, 128, SP), np.float32)
    ktf[:, :, :S] = k.transpose(0, 2, 1)
    kt = ktf.astype(ml_dtypes.bfloat16)

    # V (+ones) pre-tiled partition-major [128, T, 129]
    vv = np.zeros((H, SP, 129), np.float32)
    vv[:, :S, :128] = v
    vv[:, :TRIR, 128] = 1.0                               # ones: seg1 keys only
    vv[:, 24 * 128 + 64:] = 0.0                           # router+pad rows
    v1 = np.ascontiguousarray(
        vv.reshape(H, NKT, 128, 129).transpose(0, 2, 1, 3)).astype(ml_dtypes.bfloat16)

    v2 = np.zeros((H, 128, 4, 129), np.float32)
    for j in range(4):
        v2[:, :, j, :128] = v[:, TRI + j * 128:TRI + (j + 1) * 128]
        v2[:, :, j, 128] = 1.0
    v2 = v2.astype(ml_dtypes.bfloat16)
    v3 = np.zeros((H, 128, 17, 129), np.float32)
    for i in range(16):
        t0 = TRE + i * 128
        v3[:, :, i, :128] = v[:, t0:t0 + 128]
        v3[:, :, i, 128] = 1.0
    v3[:, 0:32, 16, :128] = v[:, TRIR:S]
    v3[:, 0:32, 16, 128] = 1.0
    v3 = v3.astype(ml_dtypes.bfloat16)

    qt2 = np.ascontiguousarray(
        (q[:, TRI:TRIR] * SQ).transpose(0, 2, 1)).astype(ml_dtypes.bfloat16)
    kt2 = np.ascontiguousarray(
        k[:, TRI:TRIR].transpose(0, 2, 1)).astype(ml_dtypes.bfloat16)

    # fused-mask payloads am16 = round(A16*mask + B16), slots (4,20..24)
    M = (rm - 1.0) * 100.0 + REF_SHIFT                    # [512, 2624]
    ref_rt = np.repeat(rt, REF // NCOND, axis=0)
    M[:, TRE:TRI] += (ref_rt - 1.0) * 100.0
    M = np.maximum(M, MASK_CLAMP)
    redux_m = np.maximum((rt - 1.0) * 100.0, MASK_CLAMP)  # [2, 2048]
    am = np.zeros((6, 128, TRI), np.float32)
    am[0, 0:32, TRE:TRI] = A16 * redux_m[0][None, :]      # tile 4 rows: redux
    am[0, 32:64, TRE:TRI] = A16 * redux_m[1][None, :]
    for tt in range(5):                                   # tiles 20..24
        blkm = np.zeros((128, TRI), np.float32)
        kk0 = (20 + tt) * 128 - TRI                       # ref-relative row
        for r in range(128):
            kr = kk0 + r
            if 0 <= kr < REF:
                blkm[r] = A16 * M[kr]
            elif kr >= REF:
                blkm[r] = A16 * MASK_CLAMP                # router+pad rows
        am[1 + tt] = blkm
    am16 = np.round(am + B16).astype(np.int16)
    am16 = np.ascontiguousarray(am16.transpose(1, 0, 2))  # [128, 6, TRI]

    in_maps = []
    for cc in range(8):
        hs = slice(HPC * cc, HPC * (cc + 1))
        in_maps.append({
            "kt": np.ascontiguousarray(kt[hs]),
            "qt": np.ascontiguousarray(qt[hs]),
            "v1": np.ascontiguousarray(v1[hs]),
            "v2": np.ascontiguousarray(v2[hs]),
            "v3": np.ascontiguousarray(v3[hs]),
            "qt2": np.ascontiguousarray(qt2[hs]),
            "kt2": np.ascontiguousarray(kt2[hs]),
            "am16": am16,
        })
    return in_maps


def kernel(query, key, value, ref_mask, routing_map, **_ignored):
    import jax
    if not any(d.platform == "axon" for d in jax.devices()):
        jax.config.update("jax_platforms", "axon,cpu")
    nc = _get_nc()
    in_maps = make_in_maps(query, key, value, ref_mask, routing_map)
    res = run_bass_kernel_spmd(nc, in_maps, core_ids=list(range(8)))
    outs = [res.results[c]["out"] for c in range(8)]      # [3, S, 129] f16
    full = np.concatenate(outs, axis=0).astype(np.float32)
    out = full[:, :, :128] / full[:, :, 128:129]
    return np.ascontiguousarray(out[None].astype(np.float32))
